# revision 30
# baseline (speedup 1.0000x reference)
"""Causal self-attention on 8 Trainium2 NeuronCores.

Problem: x[4,2048,1024], Wq/Wk/Wv/Wo[1024,1024], H=16 heads, dh=64.
    q,k,v = x@W{q,k,v}.T ; per-head causal softmax(q k^T/8) v ; out = y@Wo.T

Sharding (hybrid data+tensor parallel over 8 cores):
  core c -> (batch b = c//2, head-group hg = c%2 of 8 heads = 512 dims).
  Each core computes a partial output out_c[b] = y_hg @ Wo[:, hg].T ; the
  host sums the two partials per batch (the Wo all-reduce done on host).

Kernel layout identical to the baseline (everything transposed so no
on-device transposes are needed), but the schedule is fully software-
pipelined so the exp() work on the Activation engine overlaps matmul
work from the projection stages:

  - QKV projection matmuls for t-tile ti+1 and output-projection matmuls
    are issued as PE "filler" interleaved between attention (S / PV)
    matmul units, so the PE never idles waiting for exp.
  - Attention units are pipelined 2-deep: S(u), S(u+1) issue before
    PV(u), hiding the exp latency of unit u behind other PE work.
  - Softmax normalization: rowsum rows (from the ones-column of V') are
    gathered to lanes 32g by DVE, one wide reciprocal per q-tile, and
    the broadcast matmul reads the reciprocal directly via an f32r
    bitcast (no extract copies).  Staging copies (PSUM->SBUF) and the
    causal-mask multiplies run on the otherwise-idle Pool engine.
  - Output is bf16 (partials summed in f32 on the host), halving the
    output DMA.
"""

import sys
from collections import deque

import numpy as np

sys.path.insert(0, "/opt/trn_rl_repo")

import concourse.bass as bass  # noqa: F401
from concourse import bacc
import concourse.mybir as mybir
import concourse.tile as tile
from concourse.bass_utils import run_bass_kernel_spmd

B, T, D, H, DH = 4, 2048, 1024, 16, 64
NCORES = 8
HPC = 8                 # heads per core
JJ = HPC * DH           # 512: per-core qkv head dims
P = 128
TQ = 512                # attention q tile (free dim of S^T matmul)
TK = 128                # attention k tile (partition dim of S^T)
NDT = D // P            # 8 d-tiles (contraction for stage 1)
NJT = JJ // P           # 4 j-tiles (head-pair tiles)
NTT = T // TQ           # 4 t-tiles of 512
NKT = T // TK           # 16 k-tiles of 128
NOT_ = D // P           # 8 output row tiles (stage 3)
VW = 66                 # V row width: 64 dh + 1 ones + 1 pad
F32 = mybir.dt.float32
F32R = mybir.dt.float32r
BF16 = mybir.dt.bfloat16


def build_program():
    nc = bacc.Bacc()
    xT = nc.dram_tensor("xT", [D, T], BF16, kind="ExternalInput")
    wqT = nc.dram_tensor("wqT", [D, JJ], BF16, kind="ExternalInput")
    wkT = nc.dram_tensor("wkT", [D, JJ], BF16, kind="ExternalInput")
    wvT = nc.dram_tensor("wvT", [D, JJ], BF16, kind="ExternalInput")
    woT = nc.dram_tensor("woT", [JJ, D], BF16, kind="ExternalInput")
    maskd = nc.dram_tensor("mask", [4, P, TQ], BF16, kind="ExternalInput")
    outT = nc.dram_tensor("outT", [D, T], BF16, kind="ExternalOutput")

    xTv = xT.rearrange("(n p) t -> n p t", p=P)        # [8,128,2048]
    wqv = wqT.rearrange("(n p) j -> n p j", p=P)       # [8,128,512]
    wkv = wkT.rearrange("(n p) j -> n p j", p=P)
    wvv = wvT.rearrange("(n p) j -> n p j", p=P)
    wov = woT.rearrange("(n p) o -> n p o", p=P)       # [4,128,1024]
    outv = outT.rearrange("(n p) t -> n p t", p=P)     # [8,128,2048]

    with tile.TileContext(nc) as tc:
        with (
            tc.tile_pool(name="persist", bufs=1) as persist,
            tc.tile_pool(name="ptpool", bufs=4) as ptpool,
            tc.tile_pool(name="small", bufs=1) as small,
            tc.tile_pool(name="psMM", bufs=2, space="PSUM") as psMM,
            tc.tile_pool(name="psS", bufs=2, space="PSUM") as psS,
            tc.tile_pool(name="psY", bufs=2, space="PSUM") as psY,
        ):
            # ---- persistent SBUF tensors ----
            x_sb = persist.tile([P, NDT, T], BF16)        # all of xT
            wq_sb = persist.tile([P, NDT, JJ], BF16)
            wk_sb = persist.tile([P, NDT, JJ], BF16)
            wv_sb = persist.tile([P, NDT, JJ], BF16)
            wo_sb = persist.tile([P, NJT, D], BF16)
            qt_sb = persist.tile([P, NJT, T], BF16)       # QT [j,t]
            kt_sb = persist.tile([P, NJT, T], BF16)       # KT [j,t]
            v_sb = persist.tile([P, NKT, HPC, VW], BF16)  # V'[t, kt, h, dh|1]
            yt_sb = persist.tile([P, NJT, T], BF16)       # yT [i,t] normalized
            ystage = persist.tile([DH, HPC, TQ], F32)     # unnormalized y
            mask_sb = persist.tile([P, 4, TQ], BF16)
            ones_bf = persist.tile([P, DH], BF16)
            # rowsum staging, double-buffered on qi parity; head (g,hh)
            # gathers to partition 64*hh, free slot g (bc matmul operand
            # base partitions must be 0/64)
            rs8_d = persist.tile([P, 2, NJT, TQ], F32)

            # ones column of V' (strided memset across kt,h)
            nc.any.memset(v_sb[:, :, :, DH : DH + 1], 1.0)
            nc.any.memset(ones_bf[:], 1.0)
            nc.any.memset(rs8_d[:], 1.0)


            # ---- input DMAs: few big transfers, first-needed first ----
            # DRAM APs permuted to partition-major so both sides iterate
            # in the same order
            xTt = xT.rearrange("(n p) (f t) -> f p n t", p=P, f=NTT)
            wqp = wqT.rearrange("(n p) j -> p n j", p=P)
            wkp = wkT.rearrange("(n p) j -> p n j", p=P)
            wvp = wvT.rearrange("(n p) j -> p n j", p=P)
            wop = woT.rearrange("(n p) o -> p n o", p=P)
            maskp = maskd.rearrange("m p t -> p m t")
            nc.sync.dma_start(out=wq_sb[:], in_=wqp)
            nc.sync.dma_start(out=x_sb[:, :, 0:TQ], in_=xTt[0])
            nc.sync.dma_start(out=wk_sb[:], in_=wkp)
            nc.sync.dma_start(out=wv_sb[:], in_=wvp)
            nc.sync.dma_start(out=x_sb[:, :, TQ : 2 * TQ], in_=xTt[1])
            nc.sync.dma_start(out=mask_sb[:], in_=maskp)
            nc.sync.dma_start(out=x_sb[:, :, 2 * TQ : 3 * TQ], in_=xTt[2])
            nc.sync.dma_start(out=x_sb[:, :, 3 * TQ :], in_=xTt[3])
            nc.sync.dma_start(out=wo_sb[:], in_=wop)

            inv8 = 1.0 / float(np.sqrt(DH))

            # ================= unit generators =================

            def qk_unit(ti, w_sb, o_sb, jt):
                def run():
                    tsl = slice(ti * TQ, (ti + 1) * TQ)
                    jsl = slice(jt * P, (jt + 1) * P)
                    ps = psMM.tile([P, TQ], F32, tag="mm")
                    for dt_ in range(NDT):
                        nc.tensor.matmul(
                            ps[:],
                            lhsT=w_sb[:, dt_, jsl],
                            rhs=x_sb[:, dt_, tsl],
                            start=(dt_ == 0),
                            stop=(dt_ == NDT - 1),
                        )
                    nc.vector.tensor_copy(o_sb[:, jt, tsl], ps[:])
                return run

            def v_unit(ti, tsub):
                def run():
                    kt_idx = ti * (TQ // P) + tsub
                    ssl = slice(
                        ti * TQ + tsub * P, ti * TQ + (tsub + 1) * P
                    )
                    ps = psMM.tile([P, JJ], F32, tag="mm")
                    for dt_ in range(NDT):
                        nc.tensor.matmul(
                            ps[:],
                            lhsT=x_sb[:, dt_, ssl],
                            rhs=wv_sb[:, dt_, :],
                            start=(dt_ == 0),
                            stop=(dt_ == NDT - 1),
                        )
                    nc.vector.tensor_copy(
                        v_sb[:, kt_idx, :, 0:DH],
                        ps[:].rearrange("p (h i) -> p h i", h=HPC),
                    )
                return run

            def out_unit(ti, ot):
                def run():
                    tsl = slice(ti * TQ, (ti + 1) * TQ)
                    osl = slice(ot * P, (ot + 1) * P)
                    ps = psMM.tile([P, TQ], F32, tag="mm")
                    for it in range(NJT):
                        nc.tensor.matmul(
                            ps[:],
                            lhsT=wo_sb[:, it, osl],
                            rhs=yt_sb[:, it, tsl],
                            start=(it == 0),
                            stop=(it == NJT - 1),
                        )
                    o_sb = small.tile([P, TQ], BF16, tag="ostage", bufs=3)
                    nc.vector.tensor_copy(o_sb[:], ps[:])
                    nc.sync.dma_start(out=outv[ot][:, tsl], in_=o_sb[:])
                return run

            def qkv_units(ti):
                units = []
                for w_sb, o_sb in ((wq_sb, qt_sb), (wk_sb, kt_sb)):
                    for jt in range(NJT):
                        units.append(qk_unit(ti, w_sb, o_sb, jt))
                for tsub in range(TQ // P):
                    units.append(v_unit(ti, tsub))
                return units

            # ================= attention scheduling =================
            ucount = [0]       # global attention-unit counter
            norm_ready = deque()  # (ucount when ready, norm closure)

            def pop_norms(all_=False):
                while norm_ready and (
                    all_ or ucount[0] >= norm_ready[0][0] + 2
                ):
                    norm_ready.popleft()[1]()

            def attn(qi, fillers, early=()):
                """Issue attention for q-tile qi merged just-in-time with
                this t-tile's own QKV projection units, plus filler PE
                units (output projections of earlier tiles).  Each head
                pair's normalization issues two units after its last PV."""
                qsl = slice(qi * TQ, (qi + 1) * TQ)
                n_full = 4 * qi
                nkt = n_full + 4
                units = [(g, kt) for g in range(NJT) for kt in range(nkt)]
                nu = len(units)
                inject = {}  # unit idx -> closures to run before the unit

                def add(idx, w):
                    inject.setdefault(min(idx, nu - 1), []).append(w)

                # this tile's K/Q for head pair g, two units ahead of g
                for g in range(1, NJT):
                    add(g * nkt - 2, qk_unit(qi, wq_sb, qt_sb, g))
                    add(g * nkt - 2, qk_unit(qi, wk_sb, kt_sb, g))
                # this tile's V, before its diagonal k-tiles are reached
                for ts in range(TQ // P):
                    add(ts, v_unit(qi, ts))
                for i, w in enumerate(fillers):
                    add(int((i + 0.5) * nu / len(fillers)), w)
                # next phase's head-pair-0 projections, near the end
                for w in early:
                    add(nu - 2, w)

                rs8 = rs8_d[:, qi % 2]
                recipf = small.tile([P, NJT, TQ], F32, tag="recipf", bufs=2)
                rcomp = small.tile([P, NJT, TQ], BF16, tag="rcomp", bufs=2)
                # S / exp touch only columns q >= 128*m of a diagonal
                # k-tile m (queries before it are fully masked; kt==0 is
                # always a full tile so the PSUM 'start' covers all)
                y_ps = {}
                pend = deque()

                def issue_S(g, kt):
                    ksl = slice(kt * TK, (kt + 1) * TK)
                    m = kt - n_full
                    qo = max(m, 0) * P  # first live column of this tile
                    s2 = psS.tile([P, 2, TQ], F32, tag="att")
                    for hh in range(2):
                        hsl = slice(hh * DH, (hh + 1) * DH)
                        nc.tensor.matmul(
                            s2[:, hh, qo:],
                            lhsT=kt_sb[hsl, g, ksl],
                            rhs=qt_sb[hsl, g, qi * TQ + qo : (qi + 1) * TQ],
                            start=True,
                            stop=True,
                        )
                    pt2 = ptpool.tile([P, 2, TQ], BF16, tag="pt")
                    nc.scalar.activation(
                        pt2[:, :, qo:], s2[:, :, qo:],
                        mybir.ActivationFunctionType.Exp,
                        scale=inv8,
                    )
                    if m >= 0:
                        # diagonal tile: dead columns (q < 128m) zeroed
                        # by memset, live columns masked where k > q;
                        # PV below streams the full width.
                        if qo > 0:
                            nc.vector.memset(pt2[:, :, 0:qo], 0.0)
                        nc.vector.tensor_tensor(
                            pt2[:, :, qo:], pt2[:, :, qo:],
                            mask_sb[:, m : m + 1, qo:].to_broadcast(
                                [P, 2, TQ - qo]
                            ),
                            mybir.AluOpType.mult,
                        )
                    return pt2

                def issue_PV(g, kt, pt2):
                    for hh in range(2):
                        nc.tensor.matmul(
                            y_ps[(g, hh)][:],
                            lhsT=v_sb[:, kt, 2 * g + hh, 0 : DH + 1],
                            rhs=pt2[:, hh, :],
                            start=(kt == 0),
                            stop=(kt == nkt - 1),
                        )
                    if kt == nkt - 1:
                        # drain: y rows to ystage, rowsum row to
                        # partition 64*hh slot g of rs8 (DVE lane-shift),
                        # then this g's reciprocal + bf16 round
                        for hh in range(2):
                            h = 2 * g + hh
                            nc.vector.tensor_copy(
                                ystage[:, h, :], y_ps[(g, hh)][0:DH, :]
                            )
                            nc.vector.tensor_copy(
                                rs8[64 * hh : 64 * hh + 1, g, :],
                                y_ps[(g, hh)][DH : DH + 1, :],
                            )
                        nc.vector.reciprocal_approx_fast(
                            recipf[:, g, :], rs8[:, g, :]
                        )
                        nc.vector.tensor_copy(
                            rcomp[:, g, :], recipf[:, g, :]
                        )

                def norm_unit(g):
                    def run():
                        for hh in range(2):
                            h = 2 * g + hh
                            psl = slice(hh * DH, (hh + 1) * DH)
                            bc_ps = psMM.tile(
                                [DH, TQ], F32, tag="mm",
                                name=f"bc_{qi}_{g}_{hh}",
                            )
                            nc.tensor.matmul(
                                bc_ps[:],
                                lhsT=ones_bf[64 * hh : 64 * hh + 1, 0:DH],
                                rhs=rcomp[64 * hh : 64 * hh + 1, g, :],
                                start=True,
                                stop=True,
                            )
                            nc.vector.tensor_tensor(
                                yt_sb[psl, g, qsl],
                                ystage[:, h, :],
                                bc_ps[:],
                                mybir.AluOpType.mult,
                            )
                    return run

                def pv_front():
                    pg, pkt, ppt = pend.popleft()
                    issue_PV(pg, pkt, ppt)
                    if pkt == nkt - 1:
                        norm_ready.append((ucount[0], norm_unit(pg)))

                for idx, (g, kt) in enumerate(units):
                    for ev in inject.get(idx, ()):
                        ev()
                    if kt == 0:
                        for hh in range(2):
                            y_ps[(g, hh)] = psY.tile(
                                [DH + 1, TQ], F32, tag="y",
                                name=f"y_ps_{qi}_{g}_{hh}",
                            )
                    pt2 = issue_S(g, kt)
                    pend.append((g, kt, pt2))
                    ucount[0] += 1
                    if len(pend) >= 3:
                        pv_front()
                    pop_norms()
                while pend:
                    pv_front()

            # ================= top-level schedule =================
            # Q/K of head-pair 0 for t-tile 0 kick things off; everything
            # else is issued just-in-time inside the attention phases.
            qk_unit(0, wq_sb, qt_sb, 0)()
            qk_unit(0, wk_sb, kt_sb, 0)()

            def qk0(ti):
                return [
                    qk_unit(ti, wq_sb, qt_sb, 0),
                    qk_unit(ti, wk_sb, kt_sb, 0),
                ]

            attn(0, [], early=qk0(1))
            attn(1, [out_unit(0, ot) for ot in range(NOT_)], early=qk0(2))
            attn(2, [out_unit(1, ot) for ot in range(NOT_)], early=qk0(3))
            attn(3, [out_unit(2, ot) for ot in range(NOT_)])
            pop_norms(all_=True)
            for ot in range(NOT_):
                out_unit(3, ot)()

    nc.compile()
    return nc


def _make_masks():
    k = np.arange(P)[:, None]
    q = np.arange(TQ)[None, :]
    return np.stack(
        [(q >= k + m * P) for m in range(4)]
    ).astype(np.float32)


def make_in_maps(x, Wq, Wk, Wv, Wo):
    import ml_dtypes

    bf = ml_dtypes.bfloat16
    masks = _make_masks().astype(bf)
    x = np.asarray(x, np.float32)
    Wq, Wk, Wv, Wo = (np.asarray(w, np.float32) for w in (Wq, Wk, Wv, Wo))
    in_maps = []
    for c in range(NCORES):
        b, hg = c // 2, c % 2
        sl = slice(hg * JJ, (hg + 1) * JJ)
        in_maps.append({
            "xT": np.ascontiguousarray(x[b].T).astype(bf),
            "wqT": np.ascontiguousarray(Wq[sl].T).astype(bf),
            "wkT": np.ascontiguousarray(Wk[sl].T).astype(bf),
            "wvT": np.ascontiguousarray(Wv[sl].T).astype(bf),
            "woT": np.ascontiguousarray(Wo[:, sl].T).astype(bf),
            "mask": masks,
        })
    return in_maps


def gather_output(results):
    out = np.zeros((B, T, D), np.float32)
    for c in range(NCORES):
        out[c // 2] += np.asarray(results[c]["outT"], np.float32).T
    return out


def kernel(x, Wq, Wk, Wv, Wo):
    nc = build_program()
    in_maps = make_in_maps(x, Wq, Wk, Wv, Wo)
    res = run_bass_kernel_spmd(nc, in_maps, list(range(NCORES)))
    return gather_output(res.results)


if __name__ == "__main__":
    rng = np.random.default_rng(0)
    xs = [rng.standard_normal(s, dtype=np.float32) for s in
          [(B, T, D), (D, D), (D, D), (D, D), (D, D)]]
    out = kernel(*xs)
    print(out.shape, out.dtype)
